# revision 11
# baseline (speedup 1.0000x reference)
"""Nose-Hoover checkpointed integrator on 8 Trainium2 cores.

Data-parallel: 4096 systems sharded as 512 systems/core; each core integrates
its shard for n_steps, storing (x, v) snapshots every store_every steps.

Per-core layout: [128 partitions = systems (s mod 128), free = G groups of
64 dof], group g = s // 128 (G = 4), split into 2 phase-shifted streams of
2 groups each. Per-system scalars live as [128, 2] tiles per stream.

Math (per step, force = -x):
  beta := -(DT/2)*alpha (thermostat factor f = exp(beta)); s := -DT^2/(8Q);
  each alpha update adds s*v2 to beta plus a compile-time constant q folded
  into per-position Exp biases (renormalized once per loop iteration).

  With RT = s*v2^a and B = beta~ after update 1:
    F = exp(B + u1*q), F2 = exp(2B + 2*u1*q)
    update 2:  B2 = B + RT*F2
    TTR:       B3[:,g] = B2[:,g] + sum(s*V_g^2)  (square+reduce+update3 fused)
    F2' = exp(2*B3 + 2*u3*q), FP = exp(B3 + u3*q)
    RTc = B3 - B2;  RT' = RTc*F2'
    updates 4 and 1' fuse:  B' = B3 + 2*RT'
  V is kept without its trailing thermostat factor; GF = FP_prev * F applies
  both pending factors in one multiply before the kick-drift-kick.

Engine split per step per stream: ACT: 4 Exps. Pool: GF/RT/B updates and the
V-scale. DVE: kick-drift-kick STTs + 2 TTRs. The two streams are emitted
phase-shifted so one stream's thermostat (ACT/Pool) overlaps the other's
DVE segment.
"""

import numpy as np

DT = 0.001
N_CORES = 8
P = 128

_BUILD_CACHE = {}


def _split_multi_waits(nc, mybir):
    """This container's walrus encodes at most one sem-wait per instruction;
    hoist extra waits onto single-wait NoOps on the same engine."""
    for f in nc.m.functions:
        for bb in f.blocks:
            out = []
            for inst in bb.instructions:
                si = inst.sync_info
                if si is not None and len(si.on_wait) > 1:
                    waits = list(si.on_wait)
                    for w in waits[:-1]:
                        out.append(
                            mybir.InstNoOp(
                                name=nc.get_next_instruction_name(),
                                sync_info=mybir.SyncInfo(on_wait=[w], on_update=[]),
                                bass_nofuse=True,
                                engine=inst.engine,
                            )
                        )
                    inst.sync_info = mybir.SyncInfo(
                        on_wait=[waits[-1]], on_update=list(si.on_update)
                    )
                out.append(inst)
            bb.instructions = out


def _build_v3(
    B_core,
    D,
    n_steps,
    store_every,
    kT,
    mass,
    Q,
    bench_iters=None,
    n_streams=2,
    chunks_per_iter=4,
):
    import concourse.bass as bass
    import concourse.mybir as mybir
    from concourse.tile import TileContext

    G_total = B_core // P
    GH = G_total // n_streams
    FDH = GH * D
    n_chunks = n_steps // store_every
    assert n_steps == n_chunks * store_every
    if bench_iters is not None:
        n_chunks = chunks_per_iter
    while n_chunks % chunks_per_iter:
        chunks_per_iter -= 1
    steps_per_iter = store_every * chunks_per_iter

    k = DT / (2.0 * mass)
    e = float(D) * kT
    s = -(DT * DT) / (8.0 * Q)
    q = -s * e
    m = -DT / 2.0

    AF = mybir.ActivationFunctionType
    OP = mybir.AluOpType
    f32 = mybir.dt.float32

    nc = bass.Bass()

    def reg_const(val):
        key = (f32, float(val))
        if key not in nc.const_aps.aps:
            t = nc.alloc_sbuf_tensor(f"constb-{len(nc.const_aps.aps)}", [128, 1], f32)
            nc.gpsimd.memset(t.ap(), float(val))
            nc.const_aps.aps[key] = t.ap()

    for p_pos in range(steps_per_iter):
        for u in (4 * p_pos + 1, 4 * p_pos + 3):
            reg_const(u * q)
            reg_const(2 * u * q)
    reg_const(0.0)
    nc.all_engine_barrier()

    x0 = nc.dram_tensor("x0", [B_core, D], f32, kind="ExternalInput")
    v0 = nc.dram_tensor("v0", [B_core, D], f32, kind="ExternalInput")
    a0 = nc.dram_tensor("alpha0", [B_core], f32, kind="ExternalInput")
    n_loop_out = n_chunks // chunks_per_iter if bench_iters is None else 1
    # The dynamic-slot DMA lowering only supports a bare loop register as the
    # index, so the output is [n_loop, chunks_per_iter, ...] with ds(ci) on
    # dim 0 and the static chunk-position j on dim 1.
    out_x = nc.dram_tensor(
        "out_x",
        [n_loop_out, chunks_per_iter, G_total, P, D],
        f32,
        kind="ExternalOutput",
    )
    out_v = nc.dram_tensor(
        "out_v",
        [n_loop_out, chunks_per_iter, G_total, P, D],
        f32,
        kind="ExternalOutput",
    )

    with TileContext(nc) as tc:
        with (
            tc.tile_pool(name="state", bufs=1) as state,
            tc.tile_pool(name="stage", bufs=3) as stage,
        ):
            lanes = []
            for li in range(n_streams):
                t = lambda shape, nm: state.tile(
                    shape, f32, tag=f"{nm}{li}", name=f"{nm}{li}"
                )
                ln = {
                    "X": t([P, FDH], "X"),
                    "V": t([P, FDH], "V"),
                    "SQ": t([P, FDH], "SQ"),
                    "B": t([P, GH], "B"),
                    "B2": t([P, GH], "B2"),
                    "B3": t([P, GH], "B3"),
                    "RT": t([P, GH], "RT"),
                    "RT2": t([P, GH], "RT2"),
                    "RTC": t([P, GH], "RTC"),
                    "F": t([P, GH], "F"),
                    "F2": t([P, GH], "F2"),
                    "FP": t([P, GH], "FP"),
                    "F2P": t([P, GH], "F2P"),
                    "GF": t([P, GH], "GF"),
                    "T1": t([P, GH], "T1"),
                    "R": t([P, GH], "R"),
                    "g0": li * GH,
                }
                lanes.append(ln)

            def gsl(g):
                return slice(g * D, (g + 1) * D)

            for ln in lanes:
                g0 = ln["g0"]
                nc.sync.dma_start(
                    out=ln["X"][:].rearrange("p (g d) -> p g d", g=GH),
                    in_=x0[:].rearrange("(g p) d -> p g d", p=P)[:, g0 : g0 + GH, :],
                )
                nc.sync.dma_start(
                    out=ln["V"][:].rearrange("p (g d) -> p g d", g=GH),
                    in_=v0[:].rearrange("(g p) d -> p g d", p=P)[:, g0 : g0 + GH, :],
                )
                nc.sync.dma_start(
                    out=ln["T1"][:],
                    in_=a0[:].rearrange("(g p) -> p g", p=P)[:, g0 : g0 + GH],
                )
            for ln in lanes:
                nc.vector.tensor_scalar(ln["B2"][:], ln["T1"][:], m, None, OP.mult)
                nc.vector.memset(ln["FP"][:], 1.0)
                nc.vector.tensor_tensor(ln["SQ"][:], ln["V"][:], ln["V"][:], OP.mult)
                nc.vector.tensor_reduce(
                    out=ln["R"][:],
                    in_=ln["SQ"][:].rearrange("p (g d) -> p g d", g=GH),
                    axis=mybir.AxisListType.X,
                    op=OP.add,
                )
                nc.vector.tensor_scalar(ln["RT"][:], ln["R"][:], s, None, OP.mult)
                nc.vector.tensor_tensor(ln["B"][:], ln["RT"][:], ln["B2"][:], OP.add)

            def S1(ln, p):
                u1 = 4 * p + 1
                nc.scalar.activation(
                    out=ln["F"][:], in_=ln["B"][:], func=AF.Exp, bias=u1 * q
                )
                nc.scalar.activation(
                    out=ln["F2"][:],
                    in_=ln["B"][:],
                    func=AF.Exp,
                    scale=2.0,
                    bias=2 * u1 * q,
                )
                nc.gpsimd.tensor_tensor(ln["GF"][:], ln["FP"][:], ln["F"][:], OP.mult)
                nc.gpsimd.tensor_tensor(ln["RT2"][:], ln["RT"][:], ln["F2"][:], OP.mult)
                nc.gpsimd.tensor_tensor(ln["B2"][:], ln["B"][:], ln["RT2"][:], OP.add)

            def S2(ln, p):
                # V-scale writes into SQ (Pool rejects in-place tensor_scalar);
                # kick1 reads SQ and writes V back.
                V, X, SQ = ln["V"], ln["X"], ln["SQ"]
                for g in range(GH):
                    nc.gpsimd.tensor_scalar(
                        SQ[:, gsl(g)], V[:, gsl(g)], ln["GF"][:, g : g + 1], None, OP.mult
                    )
                nc.vector.scalar_tensor_tensor(V[:], X[:], -k, SQ[:], OP.mult, OP.add)
                nc.vector.scalar_tensor_tensor(X[:], V[:], DT, X[:], OP.mult, OP.add)
                nc.vector.scalar_tensor_tensor(V[:], X[:], -k, V[:], OP.mult, OP.add)
                nc.vector.tensor_tensor(ln["SQ"][:], V[:], V[:], OP.mult)
                nc.vector.tensor_reduce(
                    out=ln["R"][:],
                    in_=ln["SQ"][:].rearrange("p (g d) -> p g d", g=GH),
                    axis=mybir.AxisListType.X,
                    op=OP.add,
                )

            def S3(ln, p, snap_ci):
                u3 = 4 * p + 3
                # update 3: B3 = B2 + s*R  (R = sum(V^2) from S2's reduce).
                # Pool only accepts plain TT / out-of-place TSP forms, so the
                # scaled add is TSP + TT.
                nc.gpsimd.tensor_scalar(ln["RTC"][:], ln["R"][:], s, None, OP.mult)
                nc.gpsimd.tensor_tensor(ln["B3"][:], ln["RTC"][:], ln["B2"][:], OP.add)
                nc.scalar.activation(
                    out=ln["F2P"][:],
                    in_=ln["B3"][:],
                    func=AF.Exp,
                    scale=2.0,
                    bias=2 * u3 * q,
                )
                nc.scalar.activation(
                    out=ln["FP"][:], in_=ln["B3"][:], func=AF.Exp, bias=u3 * q
                )
                nc.gpsimd.tensor_tensor(ln["RT"][:], ln["RTC"][:], ln["F2P"][:], OP.mult)
                nc.vector.scalar_tensor_tensor(
                    ln["B"][:], ln["RT"][:], 2.0, ln["B3"][:], OP.mult, OP.add
                )
                if snap_ci is not None:
                    cj, slot = snap_ci
                    g0 = ln["g0"]
                    XS = stage.tile([P, FDH], f32, tag=f"XS{g0}", name=f"XS{g0}")
                    VS = stage.tile([P, FDH], f32, tag=f"VS{g0}", name=f"VS{g0}")
                    nc.gpsimd.tensor_copy(XS[:], ln["X"][:])
                    for g in range(GH):
                        nc.scalar.activation(
                            out=VS[:, gsl(g)],
                            in_=ln["V"][:, gsl(g)],
                            func=AF.Copy,
                            scale=ln["FP"][:, g : g + 1],
                        )
                    nc.sync.dma_start(
                        out=out_x[slot, cj : cj + 1, g0 : g0 + GH, :, :].rearrange(
                            "o c g p d -> (o c p) g d"
                        ),
                        in_=XS[:].rearrange("p (g d) -> p g d", g=GH),
                    )
                    nc.sync.dma_start(
                        out=out_v[slot, cj : cj + 1, g0 : g0 + GH, :, :].rearrange(
                            "o c g p d -> (o c p) g d"
                        ),
                        in_=VS[:].rearrange("p (g d) -> p g d", g=GH),
                    )

            def renorm(ln):
                nc.vector.tensor_scalar(
                    ln["B"][:], ln["B"][:], 4.0 * steps_per_iter * q, None, OP.add
                )

            n_loop = (
                n_chunks // chunks_per_iter if bench_iters is None else bench_iters
            )
            SPI = steps_per_iter

            def snap_slot(ci, p):
                if (p + 1) % store_every:
                    return None
                cj = p // store_every
                if bench_iters is None:
                    return cj, bass.ds(ci, 1)
                return cj, slice(0, 1)

            if n_loop > 0:
                with tc.For_i(
                    0, n_loop, hint_engines=(mybir.EngineType.DVE,)
                ) as ci:
                    for ln in lanes:
                        S1(ln, 0)
                    for p in range(SPI):
                        # lane li's thermostat (S3 + next S1) is emitted right
                        # after its own S2, so it executes on ACT/Pool while
                        # the next lane's S2 occupies DVE.
                        for ln in lanes:
                            S2(ln, p)
                            S3(ln, p, snap_slot(ci, p))
                            if p + 1 < SPI:
                                S1(ln, p + 1)
                            else:
                                renorm(ln)

    _split_multi_waits(nc, mybir)
    return nc


def kernel(x0, v0, alpha0, kT, mass, Q, n_steps, store_every):
    from concourse.bass_utils import run_bass_kernel_spmd

    x0 = np.asarray(x0, dtype=np.float32)
    v0 = np.asarray(v0, dtype=np.float32)
    alpha0 = np.asarray(alpha0, dtype=np.float32)
    kT_f = float(np.asarray(kT))
    mass_f = float(np.asarray(mass))
    Q_f = float(np.asarray(Q))
    n_steps = int(np.asarray(n_steps))
    store_every = int(np.asarray(store_every))

    B, D = x0.shape
    B_core = B // N_CORES
    n_chunks = n_steps // store_every

    key = (B_core, D, n_steps, store_every, kT_f, mass_f, Q_f)
    if key not in _BUILD_CACHE:
        _BUILD_CACHE[key] = _build_v3(
            B_core, D, n_steps, store_every, kT_f, mass_f, Q_f
        )
    nc = _BUILD_CACHE[key]

    in_maps = []
    for c in range(N_CORES):
        sl = slice(c * B_core, (c + 1) * B_core)
        in_maps.append(
            {
                "x0": np.ascontiguousarray(x0[sl]),
                "v0": np.ascontiguousarray(v0[sl]),
                "alpha0": np.ascontiguousarray(alpha0[sl]),
            }
        )

    res = run_bass_kernel_spmd(nc, in_maps, core_ids=list(range(N_CORES)))
    results = res.results

    traj_x = np.empty((n_chunks + 1, B, D), np.float32)
    traj_v = np.empty((n_chunks + 1, B, D), np.float32)
    traj_x[0] = x0
    traj_v[0] = v0
    for c in range(N_CORES):
        sl = slice(c * B_core, (c + 1) * B_core)
        traj_x[1:, sl] = results[c]["out_x"].reshape(n_chunks, B_core, D)
        traj_v[1:, sl] = results[c]["out_v"].reshape(n_chunks, B_core, D)
    return traj_x, traj_v
